# revision 10
# baseline (speedup 1.0000x reference)
"""CostVolume2D Trainium2 kernel.

out[b, d, h, w] = mean_c l[b,c,h,w] * r_pad[b,c,h, w + maxd - (d - maxd)]
               = mean_c l[b,c,h,w] * r[b,c,h, w - (d - maxd)]   (zero padded)

Strategy (8 NeuronCores, shard H — no halo since shifts only touch W):
  * Per (b, h): the 97 disparity planes are the diagonals of banded gram
    matrices G[w, w'] = sum_c l[c, w] r[c, w'] with |w - w'| <= 48.
  * Tensor engine computes G in [128 x 256] blocks (f32r, N=256 -> full rate):
      lhsT = l[:, w0:w0+128]  (K=64 channels on partitions)
      rhs  = r_padded[:, w0-48 : w0+208]
  * DVE evicts the needed 224 columns of each PSUM block to SBUF.
  * The skew (diagonal extraction) rides the store DMA: row i of a gram
    block holds the 97 output values for w = w0 + i *contiguously*
    (cols [i, i+97)), so a DMA with a joint partition+byte stride
    (flat stride = row_pitch + 1) writes output laid out as
    O[b, h, w, k] with k = maxd*2 - d_idx. Host unshards with a
    flip + transpose (pure layout glue).
  * Host pre-divides l by C (exact, power of two) so no on-device scaling,
    and pre-pads r along W so no on-device memset / edge handling.
"""

import sys

try:
    import concourse  # noqa: F401
except ImportError:
    sys.path.insert(0, "/opt/trn_rl_repo")

import numpy as np

from concourse import bass, mybir
from concourse import tile
from concourse.ap import AP
from concourse.bass_utils import run_bass_kernel_spmd

F32 = mybir.dt.float32
F32R = mybir.dt.float32r

# Problem dims (hardcoded per spec)
B, C, H, W = 4, 64, 256, 512
MAXD = 48
D = 2 * MAXD + 1          # 97 disparity planes
NCORES = 8
HS = H // NCORES          # 32 h-rows per core

# Derived tiling constants
WB = 128                  # w-block (gram rows per block)
NQ = W // WB              # 4 w-blocks
NMM = 256                 # matmul moving dim (>=256 for full-rate f32r)
GW = WB + 2 * MAXD        # 224 gram columns actually needed per block
RPAD_L = MAXD             # left zero pad of r
RPAD_R = NMM - WB - MAXD  # 80: right pad so q=3's 256-wide window is in bounds
WP = W + RPAD_L + RPAD_R  # 640 padded r width
HGRP = 8                  # h-rows loaded per input DMA (must divide HS, %2==0)
BW = 32                   # skew-store band width (partitions per store DMA)
KW = 2 * MAXD + BW        # 128: k-window per band
PO = 160                  # padded out row (slots); real data at [BW-1, BW-1+D)
NBAND = WB // BW          # 4

# module-level result stash (test.py reads these)
LAST_RESULTS = None
_NC_CACHE = {}


WLR = W + WP              # 1152: combined (l | r_pad) row width


def _build_nc(b_n=B, hs=HS, hgrp=HGRP):
    """Build the per-core Bass program. All cores run the same program."""
    assert hs % hgrp == 0 and hgrp % 2 == 0
    nc = bass.Bass()
    # l and r_pad concatenated on the W axis -> ONE load DMA per h-half,
    # so every matmul depends on a single DMA semaphore lane (the f32r
    # self-loading Matmult instruction only has room for one sync wait).
    lr_in = nc.dram_tensor("lr", [b_n, C, hs, WLR], F32R, kind="ExternalInput")
    o_out = nc.dram_tensor("o", [b_n, hs, NQ, WB, PO], F32, kind="ExternalOutput")

    lr_c, lr_h = hs * WLR, WLR
    lr_b = C * hs * WLR

    n4 = hgrp // 2            # h-pairs per group
    lrw = n4 * WLR            # free width of lr tile
    gp_pitch = NQ * GW        # 896: g tile row pitch

    with tile.TileContext(nc) as tc:
        with (
            tc.tile_pool(name="lrpool", bufs=2) as lrp,
            tc.tile_pool(name="gpool", bufs=4) as gp,
            tc.tile_pool(name="ppool", bufs=8, space="PSUM") as pp,
        ):
            for b in range(b_n):
                for hg in range(hs // hgrp):
                    h0 = hg * hgrp
                    lr_t = lrp.tile([128, lrw], F32R, name="lr_t")
                    # partitions = (hh in 2) x (c in 64); free = (h4, w_lr)
                    # DMA APs are limited to 3 dims -> one DMA per hh half.
                    for hh in range(2):
                        lr_src = AP(
                            lr_in, b * lr_b + (h0 + hh) * lr_h,
                            [(lr_c, C), (2 * lr_h, n4), (1, WLR)],
                        )
                        nc.sync.dma_start(
                            out=lr_t[64 * hh:64 * hh + 64, :], in_=lr_src
                        )
                    for h4 in range(n4):
                        g0 = gp.tile([128, gp_pitch], F32, name="g0", tag="g")
                        g1 = gp.tile([128, gp_pitch], F32, name="g1", tag="g")
                        gs = (g0, g1)
                        ps = {}
                        for q in range(NQ):
                            for hh in range(2):
                                p_t = pp.tile([128, NMM], F32, name="p_t")
                                lhsT = lr_t[
                                    64 * hh:64 * hh + 64,
                                    h4 * WLR + WB * q: h4 * WLR + WB * q + WB,
                                ]
                                rhs = lr_t[
                                    64 * hh:64 * hh + 64,
                                    h4 * WLR + W + WB * q:
                                    h4 * WLR + W + WB * q + NMM,
                                ]
                                nc.tensor.matmul(
                                    p_t[:], lhsT, rhs, start=True, stop=True
                                )
                                ps[(q, hh)] = p_t
                            for hh in range(2):
                                nc.vector.tensor_copy(
                                    gs[hh][:, GW * q: GW * q + GW],
                                    ps[(q, hh)][:, 0:GW],
                                )
                        for hh in range(2):
                            h = h0 + 2 * h4 + hh
                            g = gs[hh]
                            # Banded skew store: SBUF side is fully uniform
                            # (pure partition dim); the diagonal relayout is
                            # absorbed into the DRAM strides (row pitch PO
                            # minus one element per partition step). Junk
                            # columns land in padding the host strips.
                            for bb in range(NBAND):
                                i0 = BW * bb
                                s_ap = AP(
                                    g.tensor,
                                    g.offset + i0 * (gp_pitch + 1),
                                    [(gp_pitch, BW), (GW, NQ), (1, KW)],
                                )
                                d_ap = AP(
                                    o_out,
                                    (b * hs + h) * NQ * WB * PO
                                    + i0 * PO + (BW - 1),
                                    [(PO - 1, BW), (WB * PO, NQ), (1, KW)],
                                )
                                eng = nc.sync if (bb % 2) else nc.scalar
                                eng.dma_start(out=d_ap, in_=s_ap)
    _split_multi_waits(nc)
    return nc


def _split_multi_waits(nc):
    """The 64-byte TPB instruction encoding holds a single semaphore wait;
    walrus codegen rejects instructions whose sync_info carries more. Hoist
    all but one wait onto standalone InstEventSemaphore instructions placed
    immediately before, on the same engine (FIFO order preserves semantics).
    """
    for bb in nc.main_func.blocks:
        new_list = []
        changed = False
        for ins in bb.instructions:
            si = ins.sync_info
            if si is not None and len(si.on_wait) > 1:
                for w in list(si.on_wait)[:-1]:
                    ev = mybir.InstEventSemaphore(
                        name=nc.get_next_instruction_name(),
                        engine=ins.engine,
                        ins=[],
                        outs=[],
                        sync_info=mybir.SyncInfo(on_wait=[w], on_update=[]),
                    )
                    new_list.append(ev)
                ins.sync_info = mybir.SyncInfo(
                    on_wait=[list(si.on_wait)[-1]], on_update=list(si.on_update)
                )
                changed = True
            new_list.append(ins)
        if changed:
            bb.instructions = new_list


def _get_nc(key=(B, HS, HGRP)):
    if key not in _NC_CACHE:
        _NC_CACHE[key] = _build_nc(*key)
    return _NC_CACHE[key]


def _host_prep(l_fmap, r_fmap):
    l = np.asarray(l_fmap, dtype=np.float32)
    r = np.asarray(r_fmap, dtype=np.float32)
    l = l * np.float32(1.0 / C)  # exact: C is a power of two
    lr = np.empty(l.shape[:3] + (WLR,), dtype=np.float32)
    lr[..., :W] = l
    lr[..., W + RPAD_L:W + RPAD_L + W] = r
    lr[..., W:W + RPAD_L] = 0.0
    lr[..., W + RPAD_L + W:] = 0.0
    return lr


def _install_ntff_hook_shim(so_path="/opt/axon/libaxon_pjrt.so"):
    """Provide antenv.axon_hooks.get_axon_ntff_profile_hook via ctypes when
    the image's antenv lacks it (mirrors trn_agent_boot's slim hook)."""
    import types
    import ctypes
    import contextlib

    try:
        from antenv.axon_hooks import get_axon_ntff_profile_hook  # noqa: F401
        return
    except ImportError:
        pass

    lib = ctypes.CDLL(so_path)
    if not hasattr(lib, "axon_start_nrt_profile"):
        return
    lib.axon_start_nrt_profile.argtypes = [
        ctypes.POINTER(ctypes.c_int64), ctypes.c_size_t,
    ]
    lib.axon_start_nrt_profile.restype = ctypes.c_int64
    lib.axon_stop_nrt_profile.argtypes = [ctypes.c_char_p]
    lib.axon_stop_nrt_profile.restype = ctypes.c_int64

    @contextlib.contextmanager
    def _hook(output_dir, device_ids):
        import jax
        jax.devices()
        if device_ids:
            ids = (ctypes.c_int64 * len(device_ids))(*device_ids)
            rc = lib.axon_start_nrt_profile(ids, len(device_ids))
        else:
            rc = lib.axon_start_nrt_profile(None, 0)
        if rc != 0:
            raise RuntimeError(f"axon_start_nrt_profile rc={rc}")
        try:
            yield
        finally:
            n = lib.axon_stop_nrt_profile(str(output_dir).encode())
            print(f"ntff profile: {n} file(s) written to {output_dir}",
                  file=sys.stderr)

    import antenv
    mod = types.ModuleType("antenv.axon_hooks")
    mod.get_axon_ntff_profile_hook = lambda: _hook
    mod.set_axon_ntff_profile_hook = lambda h: None
    sys.modules["antenv.axon_hooks"] = mod
    antenv.axon_hooks = mod


def kernel(l_fmap, r_fmap, max_disp):
    global LAST_RESULTS
    assert int(max_disp) == MAXD
    lr = _host_prep(l_fmap, r_fmap)
    assert lr.shape == (B, C, H, WLR)

    nc = _get_nc()
    in_maps = []
    for k in range(NCORES):
        sl = slice(k * HS, (k + 1) * HS)
        in_maps.append({
            "lr": np.ascontiguousarray(lr[:, :, sl, :]),
        })

    import os
    trace = bool(int(os.environ.get("CV_TRACE", "0")))
    if trace:
        _install_ntff_hook_shim()
    res = run_bass_kernel_spmd(nc, in_maps, list(range(NCORES)), trace=trace)
    LAST_RESULTS = res

    out = np.empty((B, D, H, W), dtype=np.float32)
    for k in range(NCORES):
        o = np.asarray(res.results[k]["o"])  # [B, HS, NQ, WB, PO]
        o5 = o[..., BW - 1:BW - 1 + D]       # strip junk -> [B,HS,NQ,WB,D]
        # out[b, 96-k', h, 128q+i] = o5[b, h, q, i, k']
        tmp = np.flip(o5, axis=4).transpose(0, 4, 1, 2, 3)  # [B,D,HS,NQ,WB]
        out[:, :, k * HS:(k + 1) * HS, :] = tmp.reshape(B, D, HS, W)
    return out


# revision 11
# speedup vs baseline: 1.4280x; 1.4280x over previous
"""CostVolume2D Trainium2 kernel.

out[b, d, h, w] = mean_c l[b,c,h,w] * r_pad[b,c,h, w + maxd - (d - maxd)]
               = mean_c l[b,c,h,w] * r[b,c,h, w - (d - maxd)]   (zero padded)

Strategy (8 NeuronCores, shard H — no halo since shifts only touch W):
  * Per (b, h): the 97 disparity planes are the diagonals of banded gram
    matrices G[w, w'] = sum_c l[c, w] r[c, w'] with |w - w'| <= 48.
  * Tensor engine computes G in [128 x 256] blocks (f32r, N=256 -> full rate):
      lhsT = l[:, w0:w0+128]  (K=64 channels on partitions)
      rhs  = r_padded[:, w0-48 : w0+208]
  * DVE evicts the needed 224 columns of each PSUM block to SBUF.
  * The skew (diagonal extraction) rides the store DMA: row i of a gram
    block holds the 97 output values for w = w0 + i *contiguously*
    (cols [i, i+97)), so a DMA with a joint partition+byte stride
    (flat stride = row_pitch + 1) writes output laid out as
    O[b, h, w, k] with k = maxd*2 - d_idx. Host unshards with a
    flip + transpose (pure layout glue).
  * Host pre-divides l by C (exact, power of two) so no on-device scaling,
    and pre-pads r along W so no on-device memset / edge handling.
"""

import sys

try:
    import concourse  # noqa: F401
except ImportError:
    sys.path.insert(0, "/opt/trn_rl_repo")

import numpy as np

from concourse import bass, mybir
from concourse import tile
from concourse.ap import AP
from concourse.bass_utils import run_bass_kernel_spmd

F32 = mybir.dt.float32
F32R = mybir.dt.float32r

# Problem dims (hardcoded per spec)
B, C, H, W = 4, 64, 256, 512
MAXD = 48
D = 2 * MAXD + 1          # 97 disparity planes
NCORES = 8
HS = H // NCORES          # 32 h-rows per core

# Derived tiling constants
WB = 128                  # w-block (gram rows per block)
NQ = W // WB              # 4 w-blocks
NMM = 256                 # matmul moving dim (>=256 for full-rate f32r)
GW = WB + 2 * MAXD        # 224 gram columns actually needed per block
RPAD_L = MAXD             # left zero pad of r
RPAD_R = NMM - WB - MAXD  # 80: right pad so q=3's 256-wide window is in bounds
WP = W + RPAD_L + RPAD_R  # 640 padded r width
HGRP = 8                  # h-rows loaded per input DMA (must divide HS, %2==0)
OROW = 1024               # out slots per w-row (written with pitch OROW-1)

# module-level result stash (test.py reads these)
LAST_RESULTS = None
_NC_CACHE = {}


WLR = W + WP              # 1152: combined (l | r_pad) row width


def _build_nc(b_n=B, hs=HS, hgrp=HGRP):
    """Build the per-core Bass program. All cores run the same program."""
    assert hs % hgrp == 0 and hgrp % 2 == 0
    nc = bass.Bass()
    # l and r_pad concatenated on the W axis -> ONE load DMA per h-half,
    # so every matmul depends on a single DMA semaphore lane (the f32r
    # self-loading Matmult instruction only has room for one sync wait).
    lr_in = nc.dram_tensor("lr", [b_n, C, hs, WLR], F32R, kind="ExternalInput")
    o_out = nc.dram_tensor("o", [b_n, hs, WB, OROW], F32, kind="ExternalOutput")

    lr_c, lr_h = hs * WLR, WLR
    lr_b = C * hs * WLR

    n4 = hgrp // 2            # h-pairs per group
    lrw = n4 * WLR            # free width of lr tile
    gp_pitch = NQ * GW        # 896: g tile row pitch

    with tile.TileContext(nc) as tc:
        with (
            tc.tile_pool(name="lrpool", bufs=2) as lrp,
            tc.tile_pool(name="gpool", bufs=6) as gp,
            tc.tile_pool(name="ppool", bufs=8, space="PSUM") as pp,
        ):
            for b in range(b_n):
                for hg in range(hs // hgrp):
                    h0 = hg * hgrp
                    lr_t = lrp.tile([128, lrw], F32R, name="lr_t")
                    # partitions = (hh in 2) x (c in 64); free = (h4, w_lr)
                    # DMA APs are limited to 3 dims -> one DMA per hh half.
                    for hh in range(2):
                        lr_src = AP(
                            lr_in, b * lr_b + (h0 + hh) * lr_h,
                            [(lr_c, C), (2 * lr_h, n4), (1, WLR)],
                        )
                        nc.sync.dma_start(
                            out=lr_t[64 * hh:64 * hh + 64, :], in_=lr_src
                        )
                    for h4 in range(n4):
                        g0 = gp.tile([128, gp_pitch], F32, name="g0", tag="g")
                        g1 = gp.tile([128, gp_pitch], F32, name="g1", tag="g")
                        gs = (g0, g1)
                        ps = {}
                        for q in range(NQ):
                            for hh in range(2):
                                p_t = pp.tile([128, NMM], F32, name="p_t")
                                lhsT = lr_t[
                                    64 * hh:64 * hh + 64,
                                    h4 * WLR + WB * q: h4 * WLR + WB * q + WB,
                                ]
                                rhs = lr_t[
                                    64 * hh:64 * hh + 64,
                                    h4 * WLR + W + WB * q:
                                    h4 * WLR + W + WB * q + NMM,
                                ]
                                nc.tensor.matmul(
                                    p_t[:], lhsT, rhs, start=True, stop=True
                                )
                                ps[(q, hh)] = p_t
                            for hh in range(2):
                                nc.vector.tensor_copy(
                                    gs[hh][:, GW * q: GW * q + GW],
                                    ps[(q, hh)][:, 0:GW],
                                )
                        for hh in range(2):
                            h = h0 + 2 * h4 + hh
                            g = gs[hh]
                            # Full-row skew store: one descriptor per gram row
                            # (3584B). DRAM row pitch is OROW-1 elements, so
                            # row i's data lands shifted by -i: the diagonal
                            # relayout happens in the DRAM addressing, and the
                            # writes tile the region exactly (no overlap).
                            # Valid data sits at fixed slots 127+224q+k'.
                            d_ap = AP(
                                o_out,
                                (b * hs + h) * WB * OROW + (WB - 1),
                                [(OROW - 1, WB), (1, gp_pitch)],
                            )
                            eng = nc.sync if (hh % 2) else nc.scalar
                            eng.dma_start(out=d_ap, in_=g[:, :])
    _split_multi_waits(nc)
    return nc


def _split_multi_waits(nc):
    """The 64-byte TPB instruction encoding holds a single semaphore wait;
    walrus codegen rejects instructions whose sync_info carries more. Hoist
    all but one wait onto standalone InstEventSemaphore instructions placed
    immediately before, on the same engine (FIFO order preserves semantics).
    """
    for bb in nc.main_func.blocks:
        new_list = []
        changed = False
        for ins in bb.instructions:
            si = ins.sync_info
            if si is not None and len(si.on_wait) > 1:
                for w in list(si.on_wait)[:-1]:
                    ev = mybir.InstEventSemaphore(
                        name=nc.get_next_instruction_name(),
                        engine=ins.engine,
                        ins=[],
                        outs=[],
                        sync_info=mybir.SyncInfo(on_wait=[w], on_update=[]),
                    )
                    new_list.append(ev)
                ins.sync_info = mybir.SyncInfo(
                    on_wait=[list(si.on_wait)[-1]], on_update=list(si.on_update)
                )
                changed = True
            new_list.append(ins)
        if changed:
            bb.instructions = new_list


def _get_nc(key=(B, HS, HGRP)):
    if key not in _NC_CACHE:
        _NC_CACHE[key] = _build_nc(*key)
    return _NC_CACHE[key]


def _host_prep(l_fmap, r_fmap):
    l = np.asarray(l_fmap, dtype=np.float32)
    r = np.asarray(r_fmap, dtype=np.float32)
    l = l * np.float32(1.0 / C)  # exact: C is a power of two
    lr = np.empty(l.shape[:3] + (WLR,), dtype=np.float32)
    lr[..., :W] = l
    lr[..., W + RPAD_L:W + RPAD_L + W] = r
    lr[..., W:W + RPAD_L] = 0.0
    lr[..., W + RPAD_L + W:] = 0.0
    return lr


def _install_ntff_hook_shim(so_path="/opt/axon/libaxon_pjrt.so"):
    """Provide antenv.axon_hooks.get_axon_ntff_profile_hook via ctypes when
    the image's antenv lacks it (mirrors trn_agent_boot's slim hook)."""
    import types
    import ctypes
    import contextlib

    try:
        from antenv.axon_hooks import get_axon_ntff_profile_hook  # noqa: F401
        return
    except ImportError:
        pass

    lib = ctypes.CDLL(so_path)
    if not hasattr(lib, "axon_start_nrt_profile"):
        return
    lib.axon_start_nrt_profile.argtypes = [
        ctypes.POINTER(ctypes.c_int64), ctypes.c_size_t,
    ]
    lib.axon_start_nrt_profile.restype = ctypes.c_int64
    lib.axon_stop_nrt_profile.argtypes = [ctypes.c_char_p]
    lib.axon_stop_nrt_profile.restype = ctypes.c_int64

    @contextlib.contextmanager
    def _hook(output_dir, device_ids):
        import jax
        jax.devices()
        if device_ids:
            ids = (ctypes.c_int64 * len(device_ids))(*device_ids)
            rc = lib.axon_start_nrt_profile(ids, len(device_ids))
        else:
            rc = lib.axon_start_nrt_profile(None, 0)
        if rc != 0:
            raise RuntimeError(f"axon_start_nrt_profile rc={rc}")
        try:
            yield
        finally:
            n = lib.axon_stop_nrt_profile(str(output_dir).encode())
            print(f"ntff profile: {n} file(s) written to {output_dir}",
                  file=sys.stderr)

    import antenv
    mod = types.ModuleType("antenv.axon_hooks")
    mod.get_axon_ntff_profile_hook = lambda: _hook
    mod.set_axon_ntff_profile_hook = lambda h: None
    sys.modules["antenv.axon_hooks"] = mod
    antenv.axon_hooks = mod


def kernel(l_fmap, r_fmap, max_disp):
    global LAST_RESULTS
    assert int(max_disp) == MAXD
    lr = _host_prep(l_fmap, r_fmap)
    assert lr.shape == (B, C, H, WLR)

    nc = _get_nc()
    in_maps = []
    for k in range(NCORES):
        sl = slice(k * HS, (k + 1) * HS)
        in_maps.append({
            "lr": np.ascontiguousarray(lr[:, :, sl, :]),
        })

    import os
    trace = bool(int(os.environ.get("CV_TRACE", "0")))
    if trace:
        _install_ntff_hook_shim()
    res = run_bass_kernel_spmd(nc, in_maps, list(range(NCORES)), trace=trace)
    LAST_RESULTS = res

    out = np.empty((B, D, H, W), dtype=np.float32)
    for k in range(NCORES):
        o = np.asarray(res.results[k]["o"])  # [B, HS, WB, OROW]
        o5 = np.stack(
            [o[..., 127 + GW * q:127 + GW * q + D] for q in range(NQ)], axis=2
        )  # [B, HS, NQ, WB, D]
        # out[b, 96-k', h, 128q+i] = o5[b, h, q, i, k']
        tmp = np.flip(o5, axis=4).transpose(0, 4, 1, 2, 3)  # [B,D,HS,NQ,WB]
        out[:, :, k * HS:(k + 1) * HS, :] = tmp.reshape(B, D, HS, W)
    return out


# revision 13
# speedup vs baseline: 2.9135x; 2.0402x over previous
"""CostVolume2D Trainium2 kernel.

out[b, d, h, w] = mean_c l[b,c,h,w] * r_pad[b,c,h, w + maxd - (d - maxd)]
               = mean_c l[b,c,h,w] * r[b,c,h, w - (d - maxd)]   (zero padded)

Strategy (8 NeuronCores, shard H — no halo since shifts only touch W):
  * Per (b, h): the 97 disparity planes are the diagonals of banded gram
    matrices G[w, w'] = sum_c l[c, w] r[c, w'] with |w - w'| <= 48.
  * Tensor engine computes G in [128 x 256] blocks (f32r, N=256 -> full rate):
      lhsT = l[:, w0:w0+128]  (K=64 channels on partitions)
      rhs  = r_padded[:, w0-48 : w0+208]
  * DVE evicts the needed 224 columns of each PSUM block to SBUF.
  * The skew (diagonal extraction) rides the store DMA: row i of a gram
    block holds the 97 output values for w = w0 + i *contiguously*
    (cols [i, i+97)), so a DMA with a joint partition+byte stride
    (flat stride = row_pitch + 1) writes output laid out as
    O[b, h, w, k] with k = maxd*2 - d_idx. Host unshards with a
    flip + transpose (pure layout glue).
  * Host pre-divides l by C (exact, power of two) so no on-device scaling,
    and pre-pads r along W so no on-device memset / edge handling.
"""

import sys

try:
    import concourse  # noqa: F401
except ImportError:
    sys.path.insert(0, "/opt/trn_rl_repo")

import numpy as np

from concourse import bass, mybir
from concourse import tile
from concourse.ap import AP
from concourse.bass_utils import run_bass_kernel_spmd

F32 = mybir.dt.float32
F32R = mybir.dt.float32r
F16 = mybir.dt.float16

# Problem dims (hardcoded per spec)
B, C, H, W = 4, 64, 256, 512
MAXD = 48
D = 2 * MAXD + 1          # 97 disparity planes
NCORES = 8
HS = H // NCORES          # 32 h-rows per core

# Derived tiling constants
WB = 128                  # w-block (gram rows per block)
NQ = W // WB              # 4 w-blocks
GW = WB + 2 * MAXD        # 224 gram columns per block
NMM = GW                  # matmul moving dim (bf16: no f32r N>=256 rule)
RPAD_L = MAXD             # left zero pad of r
RPAD_R = NMM - WB - MAXD  # 48: right pad so q=3's window is in bounds
WP = W + RPAD_L + RPAD_R  # 640 padded r width
HGRP = 8                  # h-rows loaded per input DMA (must divide HS, %2==0)
OROW = 1024               # out slots per w-row (written with pitch OROW-1)

# module-level result stash (test.py reads these)
LAST_RESULTS = None
_NC_CACHE = {}


WLR = W + WP              # 1152: combined (l | r_pad) row width


def _build_nc(b_n=B, hs=HS, hgrp=HGRP):
    """Build the per-core Bass program. All cores run the same program."""
    assert hs % hgrp == 0 and hgrp % 2 == 0
    nc = bass.Bass()
    # l and r_pad concatenated on the W axis -> ONE load DMA per h-half,
    # so every matmul depends on a single DMA semaphore lane (the f32r
    # self-loading Matmult instruction only has room for one sync wait).
    lr_in = nc.dram_tensor("lr", [b_n, C, hs, WLR], F16, kind="ExternalInput")
    o_out = nc.dram_tensor("o", [b_n, hs, WB, OROW], F16, kind="ExternalOutput")

    lr_c, lr_h = hs * WLR, WLR
    lr_b = C * hs * WLR

    n4 = hgrp // 2            # h-pairs per group
    lrw = n4 * WLR            # free width of lr tile
    gp_pitch = NQ * GW        # 896: g tile row pitch

    with tile.TileContext(nc) as tc:
        with (
            tc.tile_pool(name="lrpool", bufs=2) as lrp,
            tc.tile_pool(name="gpool", bufs=6) as gp,
            tc.tile_pool(name="ppool", bufs=8, space="PSUM") as pp,
        ):
            for b in range(b_n):
                for hg in range(hs // hgrp):
                    h0 = hg * hgrp
                    lr_t = lrp.tile([128, lrw], F16, name="lr_t")
                    # partitions = (hh in 2) x (c in 64); free = (h4, w_lr)
                    # DMA APs are limited to 3 dims -> one DMA per hh half.
                    for hh in range(2):
                        lr_src = AP(
                            lr_in, b * lr_b + (h0 + hh) * lr_h,
                            [(lr_c, C), (2 * lr_h, n4), (1, WLR)],
                        )
                        nc.sync.dma_start(
                            out=lr_t[64 * hh:64 * hh + 64, :], in_=lr_src
                        )
                    for h4 in range(n4):
                        g0 = gp.tile([128, gp_pitch], F16, name="g0", tag="g")
                        g1 = gp.tile([128, gp_pitch], F16, name="g1", tag="g")
                        gs = (g0, g1)
                        for qp in range(NQ // 2):
                            for hh in range(2):
                                p_t = pp.tile([128, 2 * NMM], F32, name="p_t")
                                for qq in range(2):
                                    q = 2 * qp + qq
                                    lhsT = lr_t[
                                        64 * hh:64 * hh + 64,
                                        h4 * WLR + WB * q:
                                        h4 * WLR + WB * q + WB,
                                    ]
                                    rhs = lr_t[
                                        64 * hh:64 * hh + 64,
                                        h4 * WLR + W + WB * q:
                                        h4 * WLR + W + WB * q + NMM,
                                    ]
                                    nc.tensor.matmul(
                                        p_t[:, NMM * qq:NMM * qq + NMM],
                                        lhsT, rhs, start=True, stop=True,
                                    )
                                nc.vector.tensor_copy(
                                    gs[hh][:, 2 * GW * qp: 2 * GW * qp + 2 * GW],
                                    p_t[:],
                                )
                        for hh in range(2):
                            h = h0 + 2 * h4 + hh
                            g = gs[hh]
                            # Full-row skew store: one descriptor per gram row
                            # (3584B). DRAM row pitch is OROW-1 elements, so
                            # row i's data lands shifted by -i: the diagonal
                            # relayout happens in the DRAM addressing, and the
                            # writes tile the region exactly (no overlap).
                            # Valid data sits at fixed slots 127+224q+k'.
                            d_ap = AP(
                                o_out,
                                (b * hs + h) * WB * OROW + (WB - 1),
                                [(OROW - 1, WB), (1, gp_pitch)],
                            )
                            eng = nc.sync if (hh % 2) else nc.scalar
                            eng.dma_start(out=d_ap, in_=g[:, :])
    _split_multi_waits(nc)
    return nc


def _split_multi_waits(nc):
    """The 64-byte TPB instruction encoding holds a single semaphore wait;
    walrus codegen rejects instructions whose sync_info carries more. Hoist
    all but one wait onto standalone InstEventSemaphore instructions placed
    immediately before, on the same engine (FIFO order preserves semantics).
    """
    for bb in nc.main_func.blocks:
        new_list = []
        changed = False
        for ins in bb.instructions:
            si = ins.sync_info
            if si is not None and len(si.on_wait) > 1:
                for w in list(si.on_wait)[:-1]:
                    ev = mybir.InstEventSemaphore(
                        name=nc.get_next_instruction_name(),
                        engine=ins.engine,
                        ins=[],
                        outs=[],
                        sync_info=mybir.SyncInfo(on_wait=[w], on_update=[]),
                    )
                    new_list.append(ev)
                ins.sync_info = mybir.SyncInfo(
                    on_wait=[list(si.on_wait)[-1]], on_update=list(si.on_update)
                )
                changed = True
            new_list.append(ins)
        if changed:
            bb.instructions = new_list


def _get_nc(key=(B, HS, HGRP)):
    if key not in _NC_CACHE:
        _NC_CACHE[key] = _build_nc(*key)
    return _NC_CACHE[key]


def _host_prep(l_fmap, r_fmap):
    l = np.asarray(l_fmap, dtype=np.float32)
    r = np.asarray(r_fmap, dtype=np.float32)
    l = l * np.float32(1.0 / C)  # exact: C is a power of two
    lr = np.empty(l.shape[:3] + (WLR,), dtype=np.float16)
    lr[..., :W] = l
    lr[..., W + RPAD_L:W + RPAD_L + W] = r
    lr[..., W:W + RPAD_L] = 0.0
    lr[..., W + RPAD_L + W:] = 0.0
    return lr


def _install_ntff_hook_shim(so_path="/opt/axon/libaxon_pjrt.so"):
    """Provide antenv.axon_hooks.get_axon_ntff_profile_hook via ctypes when
    the image's antenv lacks it (mirrors trn_agent_boot's slim hook)."""
    import types
    import ctypes
    import contextlib

    try:
        from antenv.axon_hooks import get_axon_ntff_profile_hook  # noqa: F401
        return
    except ImportError:
        pass

    lib = ctypes.CDLL(so_path)
    if not hasattr(lib, "axon_start_nrt_profile"):
        return
    lib.axon_start_nrt_profile.argtypes = [
        ctypes.POINTER(ctypes.c_int64), ctypes.c_size_t,
    ]
    lib.axon_start_nrt_profile.restype = ctypes.c_int64
    lib.axon_stop_nrt_profile.argtypes = [ctypes.c_char_p]
    lib.axon_stop_nrt_profile.restype = ctypes.c_int64

    @contextlib.contextmanager
    def _hook(output_dir, device_ids):
        import jax
        jax.devices()
        if device_ids:
            ids = (ctypes.c_int64 * len(device_ids))(*device_ids)
            rc = lib.axon_start_nrt_profile(ids, len(device_ids))
        else:
            rc = lib.axon_start_nrt_profile(None, 0)
        if rc != 0:
            raise RuntimeError(f"axon_start_nrt_profile rc={rc}")
        try:
            yield
        finally:
            n = lib.axon_stop_nrt_profile(str(output_dir).encode())
            print(f"ntff profile: {n} file(s) written to {output_dir}",
                  file=sys.stderr)

    import antenv
    mod = types.ModuleType("antenv.axon_hooks")
    mod.get_axon_ntff_profile_hook = lambda: _hook
    mod.set_axon_ntff_profile_hook = lambda h: None
    sys.modules["antenv.axon_hooks"] = mod
    antenv.axon_hooks = mod


def kernel(l_fmap, r_fmap, max_disp):
    global LAST_RESULTS
    assert int(max_disp) == MAXD
    lr = _host_prep(l_fmap, r_fmap)
    assert lr.shape == (B, C, H, WLR)

    nc = _get_nc()
    in_maps = []
    for k in range(NCORES):
        sl = slice(k * HS, (k + 1) * HS)
        in_maps.append({
            "lr": np.ascontiguousarray(lr[:, :, sl, :]),
        })

    import os
    trace = bool(int(os.environ.get("CV_TRACE", "0")))
    if trace:
        _install_ntff_hook_shim()
    res = run_bass_kernel_spmd(nc, in_maps, list(range(NCORES)), trace=trace)
    LAST_RESULTS = res

    out = np.empty((B, D, H, W), dtype=np.float32)
    for k in range(NCORES):
        o = np.asarray(res.results[k]["o"]).astype(np.float32)  # [B,HS,WB,OROW]
        o5 = np.stack(
            [o[..., 127 + GW * q:127 + GW * q + D] for q in range(NQ)], axis=2
        )  # [B, HS, NQ, WB, D]
        # out[b, 96-k', h, 128q+i] = o5[b, h, q, i, k']
        tmp = np.flip(o5, axis=4).transpose(0, 4, 1, 2, 3)  # [B,D,HS,NQ,WB]
        out[:, :, k * HS:(k + 1) * HS, :] = tmp.reshape(B, D, HS, W)
    return out
